# revision 5
# baseline (speedup 1.0000x reference)
"""Trainium2 Bass kernel for DictionaryExpertLISTA.

Model: 5 LISTA layers of u = x@W[l] + z@S[l]; z = topk_mask(u, k=32 by |u|);
final recon = z @ D.T.  Returns (recon, z).

Sharding: data-parallel over batch across 8 NeuronCores (x, z sharded on
dim 0; W/S/D replicated).  No cross-core communication.

Per-core structure (B_loc=4096 rows = 32 tiles of 128):
  layer-outer loop, tile-inner.  S_l (16 MiB) + W_l (4 MiB) resident in SBUF
  per layer.  u accumulated in PSUM via fp32 matmuls (lhsT = host-transposed
  x / producer-transposed z chunks).  Top-k per row via DVE max8 +
  match_replace(-1e30) x4 rounds, then is_equal mask + multiply.  z is
  PE-transposed per tile and round-trips DRAM in transposed layout for the
  next layer's contraction.
"""
import sys
for p in ("/opt/trn_rl_repo", "/root/.axon_site/_ro/trn_rl_repo"):
    if p not in sys.path:
        sys.path.insert(0, p)

import numpy as np

N_CORES = 8
B = 32768
INPUT = 512
CODE = 2048
L = 5
K = 32
P = 128
B_LOC = B // N_CORES           # 4096
TILES = B_LOC // P             # 32
KC = INPUT // P                # 4 contract chunks for x
CC = CODE // P                 # 16 contract chunks for z
NOUT = CODE // 512             # 4 psum out chunks of 512
NEG = -1.0e30

_CACHE = {}


def _build():
    import concourse.bacc as bacc
    import concourse.mybir as mybir
    import concourse.tile as tile

    F32 = mybir.dt.float32
    ACTF = mybir.ActivationFunctionType
    ALU = mybir.AluOpType

    nc = bacc.Bacc(None, target_bir_lowering=False)

    xT = nc.declare_dram_parameter("xT", [INPUT, B_LOC], F32, isOutput=False)
    Wd = nc.declare_dram_parameter("W", [L, INPUT, CODE], F32, isOutput=False)
    Sd = nc.declare_dram_parameter("S", [L, CODE, CODE], F32, isOutput=False)
    DT = nc.declare_dram_parameter("DT", [CODE, INPUT], F32, isOutput=False)
    ident = nc.declare_dram_parameter("ident", [P, P], F32, isOutput=False)
    recon = nc.declare_dram_parameter("recon", [B_LOC, INPUT], F32, isOutput=True)
    z_out = nc.declare_dram_parameter("z_out", [B_LOC, CODE], F32, isOutput=True)

    zT_dram = nc.dram_tensor("zT_scratch", [TILES, P, CODE], F32)

    with tile.TileContext(nc) as tc:
        with tc.tile_pool(name="wpool", bufs=1) as wp, \
             tc.tile_pool(name="work", bufs=2) as wk, \
             tc.tile_pool(name="usb", bufs=1) as up, \
             tc.tile_pool(name="small", bufs=8) as sp, \
             tc.tile_pool(name="psum", bufs=2, space="PSUM") as ps:

            id_sb = wp.tile([P, P], F32, tag="ident")
            nc.sync.dma_start(out=id_sb[:], in_=ident[:])

            for l in range(L):
                # resident weights for this layer
                w_sb = wp.tile([P, KC, CODE], F32, tag="W")
                for kc in range(KC):
                    nc.sync.dma_start(
                        out=w_sb[:, kc, :],
                        in_=Wd[l, kc * P:(kc + 1) * P, :])
                if l > 0:
                    s_sb = wp.tile([P, CC, CODE], F32, tag="S")
                    for c in range(CC):
                        nc.sync.dma_start(
                            out=s_sb[:, c, :],
                            in_=Sd[l, c * P:(c + 1) * P, :])

                # transposes are software-pipelined one tile behind the
                # matmuls so the PE never waits on the DVE top-k chain.
                pending = None  # uz tile of previous batch tile

                def emit_transpose(tp, uz_prev):
                    zt_ps = ps.tile([P, CODE], F32, tag="ps")
                    for c in range(CC):
                        nc.tensor.transpose(
                            zt_ps[:, c * P:(c + 1) * P],
                            uz_prev[:, c * P:(c + 1) * P],
                            id_sb[:])
                    zt_sb = wk.tile([P, CODE], F32, tag="a")
                    nc.scalar.activation(zt_sb[:], zt_ps[:], ACTF.Copy)
                    nc.gpsimd.dma_start(out=zT_dram[tp], in_=zt_sb[:])

                for t in range(TILES):
                    # inputs for this tile
                    xt = wk.tile([P, KC, P], F32, tag="xT")
                    nc.sync.dma_start(
                        out=xt[:],
                        in_=xT[:, t * P:(t + 1) * P].rearrange(
                            "(kc p) b -> p kc b", p=P))
                    if l > 0:
                        zt_in = wk.tile([P, CC, P], F32, tag="zT")
                        nc.scalar.dma_start(
                            out=zt_in[:],
                            in_=zT_dram[t].rearrange("p (cc b) -> p cc b", b=P))

                    # u = x @ W_l (+ z @ S_l)
                    u = ps.tile([P, CODE], F32, tag="ps")
                    for n in range(NOUT):
                        ncontract = KC + (CC if l > 0 else 0)
                        ci = 0
                        for kc in range(KC):
                            nc.tensor.matmul(
                                u[:, n * 512:(n + 1) * 512],
                                xt[:, kc, :],
                                w_sb[:, kc, n * 512:(n + 1) * 512],
                                start=(ci == 0), stop=(ci == ncontract - 1))
                            ci += 1
                        if l > 0:
                            for c in range(CC):
                                nc.tensor.matmul(
                                    u[:, n * 512:(n + 1) * 512],
                                    zt_in[:, c, :],
                                    s_sb[:, c, n * 512:(n + 1) * 512],
                                    start=(ci == 0), stop=(ci == ncontract - 1))
                                ci += 1

                    # abs + copy out of PSUM (frees u's banks fast)
                    a = wk.tile([P, CODE], F32, tag="a")
                    nc.scalar.activation(a[:], u[:], ACTF.Abs)
                    uz = up.tile([P, CODE], F32, tag="u")
                    nc.scalar.activation(uz[:], u[:], ACTF.Copy)

                    # top-32 by |u|: 4 rounds max8 + match_replace
                    for r in range(4):
                        m8 = sp.tile([P, 8], F32, tag="m8")
                        nc.vector.max(m8[:], a[:])
                        nc.vector.match_replace(a[:], m8[:], a[:], NEG)
                    # mask = (a == NEG); z = u * mask   (z overwrites uz)
                    nc.vector.tensor_scalar(a[:], a[:], NEG, None, ALU.is_equal)
                    nc.vector.tensor_tensor(uz[:], uz[:], a[:], ALU.mult)

                    if l == L - 1:
                        nc.sync.dma_start(
                            out=z_out[t * P:(t + 1) * P, :], in_=uz[:])

                    if pending is not None:
                        emit_transpose(t - 1, pending)
                    pending = uz
                emit_transpose(TILES - 1, pending)

            # recon = z @ D.T
            dt_sb = wp.tile([P, CC, INPUT], F32, tag="S")
            for c in range(CC):
                nc.sync.dma_start(
                    out=dt_sb[:, c, :], in_=DT[c * P:(c + 1) * P, :])
            for t in range(TILES):
                zt_in = wk.tile([P, CC, P], F32, tag="zT")
                nc.scalar.dma_start(
                    out=zt_in[:],
                    in_=zT_dram[t].rearrange("p (cc b) -> p cc b", b=P))
                r_ps = ps.tile([P, INPUT], F32, tag="ps")
                for c in range(CC):
                    nc.tensor.matmul(
                        r_ps[:], zt_in[:, c, :], dt_sb[:, c, :],
                        start=(c == 0), stop=(c == CC - 1))
                r_sb = up.tile([P, INPUT], F32, tag="u")
                nc.scalar.activation(r_sb[:], r_ps[:], ACTF.Copy)
                nc.sync.dma_start(out=recon[t * P:(t + 1) * P, :], in_=r_sb[:])

    nc.compile()
    return nc


def get_nc():
    if "nc" not in _CACHE:
        _CACHE["nc"] = _build()
    return _CACHE["nc"]


def kernel(x, W, S, D):
    from concourse.bass_utils import run_bass_kernel_spmd

    nc = get_nc()
    x = np.ascontiguousarray(x, dtype=np.float32)
    W = np.ascontiguousarray(W, dtype=np.float32)
    S = np.ascontiguousarray(S, dtype=np.float32)
    D = np.ascontiguousarray(D, dtype=np.float32)

    xT = np.ascontiguousarray(x.T)                 # [INPUT, B]
    DTm = np.ascontiguousarray(D.T)                # [CODE, INPUT]
    ident = np.eye(P, dtype=np.float32)

    in_maps = []
    for c in range(N_CORES):
        in_maps.append(dict(
            xT=np.ascontiguousarray(xT[:, c * B_LOC:(c + 1) * B_LOC]),
            W=W, S=S, DT=DTm, ident=ident,
        ))
    res = run_bass_kernel_spmd(nc, in_maps, list(range(N_CORES)))
    recon = np.concatenate([r["recon"] for r in res.results], axis=0)
    z = np.concatenate([r["z_out"] for r in res.results], axis=0)
    return recon, z


# revision 23
# speedup vs baseline: 13.6141x; 13.6141x over previous
"""Trainium2 Bass kernel for DictionaryExpertLISTA.

Model: 5 LISTA layers of u = x@W[l] + z@S[l]; z = topk_mask(u, k=32 by |u|);
final recon = z @ D.T.  Returns (recon, z).

Sharding: data-parallel over batch across 8 NeuronCores (x, z sharded on
dim 0; W/S/D replicated).  No cross-core communication.

Matmul precision: 3-term fp16 split (v = vh + vl/2048, vh = fp16(v),
vl = fp16(2048*(v - vh)); u = vh@Wh + (vh@Wl' + vl'@Wh)/2048).  fp16
products are exact in fp32 PSUM accumulation, so this matches native fp32
matmul accuracy (verified on HW: closer to fp64 than numpy fp32) at
1 cycle/row instead of fp32's 4.

Per-core structure (B_loc=4096 rows = 32 tiles of 128): layer-outer loop,
tile-inner, S_l/W_l resident in SBUF (fp16 hi+lo pairs).  Top-k per row via
DVE max8 + match_replace(-1e30) x4, then is_equal mask + multiply (gpsimd).
z is PE-transposed (scaled-identity x2048), split to fp16 hi/lo on-chip, and
round-trips DRAM in transposed fp16 layout for the next layer's contraction.
Transposes are software-pipelined one tile behind the matmuls so the PE
never waits on the DVE top-k chain.
"""
import sys
for p in ("/opt/trn_rl_repo", "/root/.axon_site/_ro/trn_rl_repo"):
    if p not in sys.path:
        sys.path.insert(0, p)

import numpy as np

N_CORES = 8
B = 32768
INPUT = 512
CODE = 2048
L = 5
K = 32
P = 128
B_LOC = B // N_CORES           # 4096
TILES = B_LOC // P             # 32
KC = INPUT // P                # 4 contract chunks for x
CC = CODE // P                 # 16 contract chunks for z
NEG = -1.0e30
SC = 2048.0                    # 2^11 lo-term scale

_CACHE = {}


def _build():
    import concourse.bacc as bacc
    import concourse.mybir as mybir
    import concourse.tile as tile

    F32 = mybir.dt.float32
    F16 = mybir.dt.float16
    ACTF = mybir.ActivationFunctionType
    ALU = mybir.AluOpType

    nc = bacc.Bacc(None, target_bir_lowering=False)

    xTh = nc.declare_dram_parameter("xTh", [INPUT, B_LOC], F16, isOutput=False)
    xTl = nc.declare_dram_parameter("xTl", [INPUT, B_LOC], F16, isOutput=False)
    Wh = nc.declare_dram_parameter("Wh", [L, INPUT, CODE], F16, isOutput=False)
    Wl = nc.declare_dram_parameter("Wl", [L, INPUT, CODE], F16, isOutput=False)
    Sh = nc.declare_dram_parameter("Sh", [L, CODE, CODE], F16, isOutput=False)
    Sl = nc.declare_dram_parameter("Sl", [L, CODE, CODE], F16, isOutput=False)
    DTh = nc.declare_dram_parameter("DTh", [CODE, INPUT], F16, isOutput=False)
    DTl = nc.declare_dram_parameter("DTl", [CODE, INPUT], F16, isOutput=False)
    id2k = nc.declare_dram_parameter("id2k", [P, P], F32, isOutput=False)
    recon = nc.declare_dram_parameter("recon", [B_LOC, INPUT], F32, isOutput=True)
    z_out = nc.declare_dram_parameter("z_out", [B_LOC, CODE], F32, isOutput=True)

    zTh_dram = nc.dram_tensor("zTh_scratch", [TILES, P, CODE], F16)
    zTl_dram = nc.dram_tensor("zTl_scratch", [TILES, P, CODE], F16)

    H = 1024  # out-half width (fp16 moving max)

    with tile.TileContext(nc) as tc:
        with tc.tile_pool(name="wpool", bufs=1) as wp, \
             tc.tile_pool(name="work", bufs=2) as wk, \
             tc.tile_pool(name="zin", bufs=1) as zi, \
             tc.tile_pool(name="apool", bufs=1) as ap_pool, \
             tc.tile_pool(name="usb", bufs=2) as up, \
             tc.tile_pool(name="zsplit", bufs=1) as zs, \
             tc.tile_pool(name="small", bufs=4) as sp, \
             tc.tile_pool(name="psum", bufs=2, space="PSUM") as ps:

            id_sb = wp.tile([P, P], F32, tag="ident")
            nc.sync.dma_start(out=id_sb[:], in_=id2k[:])

            for l in range(L):
                wh_sb = wp.tile([P, KC, CODE], F16, tag="Wh")
                wl_sb = wp.tile([P, KC, CODE], F16, tag="Wl")
                for kc in range(KC):
                    nc.sync.dma_start(out=wh_sb[:, kc, :],
                                      in_=Wh[l, kc * P:(kc + 1) * P, :])
                    nc.sync.dma_start(out=wl_sb[:, kc, :],
                                      in_=Wl[l, kc * P:(kc + 1) * P, :])
                if l > 0:
                    sh_sb = wp.tile([P, CC, CODE], F16, tag="Sh")
                    sl_sb = wp.tile([P, CC, CODE], F16, tag="Sl")
                    for c in range(CC):
                        nc.sync.dma_start(out=sh_sb[:, c, :],
                                          in_=Sh[l, c * P:(c + 1) * P, :])
                        nc.sync.dma_start(out=sl_sb[:, c, :],
                                          in_=Sl[l, c * P:(c + 1) * P, :])

                pending = None  # uz of previous tile, for pipelined transpose

                def emit_transpose(tp, uz_prev, scaled):
                    # zt_ps = (2048*z).T for scaled layers (the 2048 rides on
                    # the top-k mask — the HW transpose ignores identity
                    # values, so it can't scale).  hi' = fp16(2048*z) is
                    # descaled by 2^-11 (exact) for the main group; lo' =
                    # 2048*lo stays normal-range fp16 (no subnormal FTZ).
                    # The last layer (recon consumer) is unscaled: its lo
                    # subnormal tail only perturbs recon by ~1e-6.
                    zt_ps = ps.tile([P, CODE], F32, tag="ps")
                    for c in range(CC):
                        nc.tensor.transpose(
                            zt_ps[:, c * P:(c + 1) * P],
                            uz_prev[:, c * P:(c + 1) * P],
                            id_sb[:])
                    hi = zs.tile([P, CODE], F16, tag="hi")
                    lo = zs.tile([P, CODE], F16, tag="lo")
                    nc.scalar.activation(hi[:], zt_ps[:], ACTF.Copy)
                    nc.vector.tensor_tensor(lo[:], zt_ps[:], hi[:], ALU.subtract)
                    nc.gpsimd.dma_start(out=zTl_dram[tp], in_=lo[:])
                    if scaled:
                        nc.gpsimd.tensor_scalar(hi[:], hi[:], 1.0 / SC, None,
                                                ALU.mult)
                    nc.gpsimd.dma_start(out=zTh_dram[tp], in_=hi[:])

                for t in range(TILES):
                    xth = wk.tile([P, KC, P], F16, tag="xTh")
                    xtl = wk.tile([P, KC, P], F16, tag="xTl")
                    for src, dst in ((xTh, xth), (xTl, xtl)):
                        nc.sync.dma_start(
                            out=dst[:],
                            in_=src[:, t * P:(t + 1) * P].rearrange(
                                "(kc p) b -> p kc b", p=P))
                    if l > 0:
                        # half-tiles (chunks 0-7 / 8-15), bufs=1: the first
                        # half's reload overlaps the second half's use
                        zparts = {}
                        for nm, dram in (("h", zTh_dram), ("l", zTl_dram)):
                            for half in range(2):
                                zt_part = zi.tile([P, CC // 2, P], F16,
                                                  tag=f"zT{nm}{half}")
                                nc.scalar.dma_start(
                                    out=zt_part[:],
                                    in_=dram[t, :, half * (CODE // 2):
                                             (half + 1) * (CODE // 2)].rearrange(
                                                 "p (cc b) -> p cc b", b=P))
                                zparts[(nm, half)] = zt_part

                        def zth_c(c):
                            return zparts[("h", c // 8)][:, c % 8, :]

                        def ztl_c(c):
                            return zparts[("l", c // 8)][:, c % 8, :]

                    uz = up.tile([P, CODE], F32, tag="u")
                    for h in range(2):
                        hs = slice(h * H, (h + 1) * H)
                        pm = ps.tile([P, CODE], F32, tag="ps")
                        main, corr = pm[:, 0:H], pm[:, H:2 * H]
                        # ops grouped by stationary so consecutive matmuls
                        # reuse the loaded weights: (stationary, [(region,
                        # moving), ...])
                        ops = [(xth[:, kc, :], [("m", wh_sb[:, kc, hs]),
                                                ("c", wl_sb[:, kc, hs])])
                               for kc in range(KC)]
                        ops += [(xtl[:, kc, :], [("c", wh_sb[:, kc, hs])])
                                for kc in range(KC)]
                        if l > 0:
                            # producer scaled lo by 2048 -> zl@Sh is corr
                            ops += [(zth_c(c), [("m", sh_sb[:, c, hs]),
                                                ("c", sl_sb[:, c, hs])])
                                    for c in range(CC)]
                            ops += [(ztl_c(c), [("c", sh_sb[:, c, hs])])
                                    for c in range(CC)]
                        n_m = sum(1 for _, ms in ops for r, _ in ms if r == "m")
                        n_c = sum(1 for _, ms in ops for r, _ in ms if r == "c")
                        cnt = {"m": 0, "c": 0}
                        tot = {"m": n_m, "c": n_c}
                        reg = {"m": main, "c": corr}
                        # fp16 moving operand max is 512 wide
                        for st, movs in ops:
                            for r, mv in movs:
                                i = cnt[r]
                                cnt[r] += 1
                                for s in range(2):
                                    nc.tensor.matmul(
                                        reg[r][:, s * 512:(s + 1) * 512], st,
                                        mv[:, s * 512:(s + 1) * 512],
                                        start=(i == 0), stop=(i == tot[r] - 1))
                        # u = corr/2048 + main
                        nc.scalar.activation(uz[:, hs], corr, ACTF.Copy,
                                             scale=1.0 / SC)
                        nc.vector.tensor_tensor(uz[:, hs], uz[:, hs], main,
                                                ALU.add)

                    # top-32 by |u|
                    a = ap_pool.tile([P, CODE], F32, tag="a")
                    nc.scalar.activation(a[:], uz[:], ACTF.Abs)
                    for r in range(4):
                        m8 = sp.tile([P, 8], F32, tag="m8")
                        nc.vector.max(m8[:], a[:])
                        nc.vector.match_replace(a[:], m8[:], a[:], NEG)
                    # mask: 0/2048 for scaled layers (z arrives pre-scaled at
                    # the transpose), plain 0/1 for the last layer
                    if l < L - 1:
                        nc.gpsimd.tensor_scalar(a[:], a[:], NEG, SC,
                                                ALU.is_equal, ALU.mult)
                    else:
                        nc.gpsimd.tensor_scalar(a[:], a[:], NEG, None,
                                                ALU.is_equal)
                    nc.gpsimd.tensor_tensor(uz[:], uz[:], a[:], ALU.mult)

                    if l == L - 1:
                        nc.sync.dma_start(
                            out=z_out[t * P:(t + 1) * P, :], in_=uz[:])

                    if pending is not None:
                        emit_transpose(t - 1, pending, l < L - 1)
                    pending = uz
                emit_transpose(TILES - 1, pending, l < L - 1)

            # recon = z @ D.T, same 3-term fp16 scheme
            dth_sb = wp.tile([P, CC, INPUT], F16, tag="Sh")
            dtl_sb = wp.tile([P, CC, INPUT], F16, tag="Sl")
            for c in range(CC):
                nc.sync.dma_start(out=dth_sb[:, c, :], in_=DTh[c * P:(c + 1) * P, :])
                nc.sync.dma_start(out=dtl_sb[:, c, :], in_=DTl[c * P:(c + 1) * P, :])
            for t in range(TILES):
                zparts = {}
                for nm, dram in (("h", zTh_dram), ("l", zTl_dram)):
                    for half in range(2):
                        zt_part = zi.tile([P, CC // 2, P], F16,
                                          tag=f"zT{nm}{half}")
                        nc.scalar.dma_start(
                            out=zt_part[:],
                            in_=dram[t, :, half * (CODE // 2):
                                     (half + 1) * (CODE // 2)].rearrange(
                                         "p (cc b) -> p cc b", b=P))
                        zparts[(nm, half)] = zt_part
                zth_c = lambda c: zparts[("h", c // 8)][:, c % 8, :]
                ztl_c = lambda c: zparts[("l", c // 8)][:, c % 8, :]
                pm = ps.tile([P, CODE], F32, tag="ps")
                main, corr = pm[:, 0:INPUT], pm[:, INPUT:2 * INPUT]
                ops = [(zth_c(c), [("m", dth_sb[:, c, :]),
                                   ("c", dtl_sb[:, c, :])]) for c in range(CC)]
                ops += [(ztl_c(c), [("m", dth_sb[:, c, :])]) for c in range(CC)]
                cnt = {"m": 0, "c": 0}
                tot = {"m": 2 * CC, "c": CC}
                reg = {"m": main, "c": corr}
                for st, movs in ops:
                    for r, mv in movs:
                        i = cnt[r]
                        cnt[r] += 1
                        nc.tensor.matmul(reg[r], st, mv, start=(i == 0),
                                         stop=(i == tot[r] - 1))
                r_sb = up.tile([P, INPUT], F32, tag="u")
                nc.scalar.activation(r_sb[:], corr, ACTF.Copy, scale=1.0 / SC)
                nc.vector.tensor_tensor(r_sb[:], r_sb[:], main, ALU.add)
                nc.sync.dma_start(out=recon[t * P:(t + 1) * P, :], in_=r_sb[:])

    nc.compile()
    return nc


def get_nc():
    if "nc" not in _CACHE:
        _CACHE["nc"] = _build()
    return _CACHE["nc"]


def _split16(v):
    hi = v.astype(np.float16)
    lo = ((v - hi.astype(np.float32)) * np.float32(SC)).astype(np.float16)
    return hi, lo


def make_in_maps(x, W, S, D):
    x = np.ascontiguousarray(x, dtype=np.float32)
    W = np.ascontiguousarray(W, dtype=np.float32)
    S = np.ascontiguousarray(S, dtype=np.float32)
    D = np.ascontiguousarray(D, dtype=np.float32)

    xT = np.ascontiguousarray(x.T)
    xTh, xTl = _split16(xT)
    Wh, Wl = _split16(W)
    Sh, Sl = _split16(S)
    DTh, DTl = _split16(np.ascontiguousarray(D.T))
    id2k = np.eye(P, dtype=np.float32)

    in_maps = []
    for c in range(N_CORES):
        sl_ = slice(c * B_LOC, (c + 1) * B_LOC)
        in_maps.append(dict(
            xTh=np.ascontiguousarray(xTh[:, sl_]),
            xTl=np.ascontiguousarray(xTl[:, sl_]),
            Wh=Wh, Wl=Wl, Sh=Sh, Sl=Sl, DTh=DTh, DTl=DTl, id2k=id2k,
        ))
    return in_maps


def kernel(x, W, S, D):
    from concourse.bass_utils import run_bass_kernel_spmd

    nc = get_nc()
    in_maps = make_in_maps(x, W, S, D)
    res = run_bass_kernel_spmd(nc, in_maps, list(range(N_CORES)))
    recon = np.concatenate([r["recon"] for r in res.results], axis=0)
    z = np.concatenate([r["z_out"] for r in res.results], axis=0)
    return recon, z
